# revision 23
# baseline (speedup 1.0000x reference)
"""LSTM cell (batch 8192, input 512, hidden 512) on 8 Trainium2 NeuronCores.

Data-parallel over the batch dim: each core handles 1024 rows. Weights are
replicated. The host pre-transposes both matmul operands so the contraction
dim (fan_in = 1024) lands on SBUF partitions:

  gate.T[n, b] = sum_k W.T[k, n] * combined.T[k, b]     (matmul: lhsT.T @ rhs)

so the kernel computes everything in [hidden, batch] layout; gate biases
become per-partition vectors (free on the ACT activation op), and the host
transposes the outputs back after the gather.

The kernel is PE-bound, so precision is spent where the accuracy budget
allows:
 - f/c/o gate matmuls are bf16 (their fp8 error blows the 2e-2 gate: the
   f-gate error is amplified by c_prev, the c~-gate error passes straight
   through tanh into c_next).
 - The i-gate matmul for h>=1 runs fp8-e4m3 in DoubleRow mode (2 MACs per
   cell per cycle), halving its PE time. i only multiplies the bounded c~,
   so its fp8 error lands at ~1.7e-2 end to end (verified against the
   reference). Both operands are pre-scaled by 16 on the host to stay
   clear of the e4m3 subnormal region; the ACT sigmoid's free affine
   scale (1/256) undoes it. The fp8<->bf16 PE array mode switch costs a
   ~0.4us pipeline bubble, so all i-gate matmuls of one h-chunk are
   batched (one switch pair per h). The h=0 i-gate stays bf16: the h=0
   phase is DMA-bound and adding the fp8 operand stream to the ramp costs
   more than the fp8 matmul saves.
 - The elementwise tail, c_prev, and the outputs are bf16 (2x DVE rate,
   half the HBM traffic); the host widens outputs back to f32.

DMA scheduling is issuance-budgeted: a dma_start occupies its issuing
engine for ~0.65us, so the h=0 phase uses fine-grained per-k strips
(paced against the matmul ramp) while the h=1..3 weights arrive as one
contiguous block per h. Dummy matmuls on a memset tile pre-warm the PE's
HAM clock gate during the ramp. Outputs ride the gpsimd SWDGE ring except
the final h-chunk's, which use sync so the exit barrier isn't waiting on
a late SWDGE receipt.
"""

import numpy as np

import concourse.bacc as bacc
import concourse.bass as bass
import concourse.mybir as mybir
from concourse import tile
from concourse.bass_utils import run_bass_kernel_spmd

N_CORES = 8
BATCH = 8192
B = BATCH // N_CORES  # 1024 batch rows per core
K = 1024              # fan_in = input_dim + hidden_dim
H = 512               # hidden dim
NG = 4                # gates: i, f, c, o
KT = K // 128         # 8 contraction tiles
KP = KT // 2          # 4 fp8 DoubleRow k-pair tiles
HT = H // 128         # 4 hidden chunks per gate
BT = B // 512         # 2 batch halves (PSUM free-dim limit is 512 f32)

MM_DT = mybir.dt.bfloat16
FP8 = mybir.dt.float8e4
F32 = mybir.dt.float32
FP8_SCALE = 16.0      # host-side pre-scale on both i-gate operands

_SIG = mybir.ActivationFunctionType.Sigmoid
_TANH = mybir.ActivationFunctionType.Tanh
_DR = mybir.MatmulPerfMode.DoubleRow
# gate order: i, f, c, o
_GATE_FN = [_SIG, _SIG, _TANH, _SIG]


def _build():
    nc = bacc.Bacc(
        "TRN2",
        target_bir_lowering=False,
        debug=False,
        num_devices=N_CORES,
    )

    xhT = nc.dram_tensor("xhT", [K, B], MM_DT, kind="ExternalInput")
    # fp8 activations, k-pair-interleaved: row kp*128+p, col s*B+b holds
    # 16*combined.T[(2kp+s)*128+p, b] so each pair tile is contiguous.
    xh8P = nc.dram_tensor("xh8P", [KP * 128, 2 * B], FP8, kind="ExternalInput")
    # h=0 weights, all four gates, one contiguous 128KB strip per k:
    # row k*128+p, col g*128+c.
    w0P = nc.dram_tensor("w0P", [KT * 128, NG * 128], MM_DT, kind="ExternalInput")
    # h>=1 f/c/o weights, one contiguous 768KB block per h:
    # row (h-1)*128+p, col k*384 + (g-1)*128 + c.
    wHP = nc.dram_tensor(
        "wHP", [(HT - 1) * 128, KT * 3 * 128], MM_DT, kind="ExternalInput",
    )
    # h>=1 fp8 i-gate weights, one contiguous 128KB block per h:
    # row (h-1)*128+p, col kp*256 + s*128 + c.
    wi8H = nc.dram_tensor(
        "wi8H", [(HT - 1) * 128, KP * 256], FP8, kind="ExternalInput",
    )
    bias2d = nc.dram_tensor("bias2d", [128, NG * HT], F32, kind="ExternalInput")
    c_prevT = nc.dram_tensor("c_prevT", [H, B], MM_DT, kind="ExternalInput")
    h_nextT = nc.dram_tensor("h_nextT", [H, B], MM_DT, kind="ExternalOutput")
    c_nextT = nc.dram_tensor("c_nextT", [H, B], MM_DT, kind="ExternalOutput")

    with tile.TileContext(nc) as tc:
        with (
            tc.tile_pool(name="wts", bufs=1) as wpool,
            tc.tile_pool(name="acts", bufs=1) as apool,
            tc.tile_pool(name="cprev", bufs=1) as cpool,
            tc.tile_pool(name="gates", bufs=3) as gpool,
            tc.tile_pool(name="ew", bufs=3) as epool,
            tc.tile_pool(name="psum", bufs=1, space="PSUM") as pspool,
        ):
            xh_tiles = [None] * KT          # [k] -> [128, B] bf16 (k>=1)
            xh0_half = [None, None]         # k=0 b2 halves [128, 512]
            x8_tiles = [None] * KP          # [kp] -> [128, 2, B] fp8
            w0_tiles = [None] * KT          # [k] -> [128, 512] bf16 (h=0)
            wH_tiles = [None] * HT          # [h] -> [128, KT*384] bf16 (h>=1)
            wi_tiles = [None] * HT          # [h] -> [128, KP*2, 128] fp8
            cp_tiles = [None] * HT

            def _load_w0(k, eng=None):
                wt = wpool.tile([128, NG * 128], MM_DT, tag=f"w0_{k}", name=f"w0_{k}")
                (eng or nc.sync).dma_start(wt[:], w0P[k * 128:(k + 1) * 128, :])
                w0_tiles[k] = wt

            def _load_wH(h):
                wt = wpool.tile([128, KT * 384], MM_DT, tag=f"wH{h}", name=f"wH{h}")
                nc.sync.dma_start(wt[:], wHP[(h - 1) * 128:h * 128, :])
                wH_tiles[h] = wt

            def _load_wi(h):
                wt = wpool.tile(
                    [128, KP * 2, 128], FP8, tag=f"wi{h}", name=f"wi{h}",
                )
                nc.sync.dma_start(wt[:], wi8H[(h - 1) * 128:h * 128, :])
                wi_tiles[h] = wt

            def _load_x8(kp):
                xt = apool.tile([128, 2, B], FP8, tag=f"x8_{kp}", name=f"x8_{kp}")
                nc.scalar.dma_start(xt[:], xh8P[kp * 128:(kp + 1) * 128, :])
                x8_tiles[kp] = xt

            def _load_cp(h):
                ct = cpool.tile([128, B], MM_DT, tag=f"cp{h}", name=f"cp{h}")
                nc.sync.dma_start(ct[:], c_prevT[h * 128:(h + 1) * 128, :])
                cp_tiles[h] = ct

            # ---- input DMA schedule ------------------------------------
            # scalar ring: k=0/1 h=0 weight strips (so the first matmul's
            #   two dependencies land in parallel), then the fp8 activation
            #   pair tiles (first used at h=1).
            # sync ring: h=0 k-stream (weights + activations, k-major,
            #   paced against the matmul ramp), with the h=1..3 per-h
            #   weight blocks and c_prev tiles threaded through at the
            #   points where issuance slack exists.
            _load_w0(0, eng=nc.scalar)
            _load_w0(1, eng=nc.scalar)
            for kp in range(KP):
                _load_x8(kp)

            for b2 in range(BT):
                xt = apool.tile(
                    [128, 512], MM_DT, tag=f"xh0_{b2}", name=f"xh0_{b2}",
                )
                nc.sync.dma_start(xt[:], xhT[0:128, b2 * 512:(b2 + 1) * 512])
                xh0_half[b2] = xt
            bias_t = wpool.tile([128, NG * HT], F32, tag="bias", name="bias_t")
            nc.sync.dma_start(bias_t[:], bias2d[:])
            for k in range(1, KT):
                xt = apool.tile([128, B], MM_DT, tag=f"xh{k}", name=f"xh{k}")
                nc.sync.dma_start(xt[:], xhT[k * 128:(k + 1) * 128, :])
                xh_tiles[k] = xt
                if k >= 2:
                    _load_w0(k)
                if k == 3:
                    _load_wi(1)
                    _load_wH(1)
                if k == 5:
                    _load_cp(0)
            _load_wi(2)
            _load_wH(2)
            _load_cp(1)
            _load_wi(3)
            _load_wH(3)
            _load_cp(2)
            _load_cp(3)

            # PE pre-warm: the HAM clock gate holds the PE at 1.2 GHz until
            # it has seen ~3.4us of sustained matmul activity. Dummy matmuls
            # on a memset tile during the input-DMA ramp start that clock
            # early, so the real matmul stream runs warm (2.4 GHz) almost
            # from its first instruction.
            warm_t = apool.tile([128, 512], MM_DT, tag="warm", name="warm_t")
            nc.vector.memset(warm_t[:], 0)
            warm_ps = pspool.tile([128, 512], F32, tag="ps0_0", name="warm_ps")
            for _ in range(5):
                nc.tensor.matmul(
                    warm_ps[:], warm_t[:, 0:128], warm_t[:],
                    start=True, stop=True,
                )

            def _rhs(k, b2):
                if k == 0:
                    return xh0_half[b2][:]
                return xh_tiles[k][:, b2 * 512:(b2 + 1) * 512]

            def _rhs8(kp, b2):
                return x8_tiles[kp][:, :, b2 * 512:(b2 + 1) * 512]

            def _lhsT(k, h, g):
                if h == 0:
                    return w0_tiles[k][:, g * 128:(g + 1) * 128]
                # bf16 gates only for h>=1: g in {1, 2, 3}
                off = k * 384 + (g - 1) * 128
                return wH_tiles[h][:, off:off + 128]

            def _mm_i(psum_i, kp, h, b2):
                nc.tensor.matmul(
                    psum_i,
                    wi_tiles[h][:, 2 * kp:2 * kp + 2, :],
                    _rhs8(kp, b2),
                    start=(kp == 0),
                    stop=(kp == KP - 1),
                    perf_mode=_DR,
                )

            def _mk_psum(g, h, b2):
                return pspool.tile(
                    [128, 512], F32,
                    tag=f"ps{g}_{b2 % 2}", name=f"ps{g}_{h}_{b2}",
                )

            def _elementwise(h, b2, psum):
                """Activations + LSTM cell tail for one (h, b2) group."""
                hs = slice(h * 128, (h + 1) * 128)
                cs = slice(b2 * 512, (b2 + 1) * 512)

                def _act_gate(g):
                    t = gpool.tile(
                        [128, 512], MM_DT, tag=f"g{g}", name=f"g{g}_{h}_{b2}",
                    )
                    nc.scalar.activation(
                        t[:], psum[g][:], _GATE_FN[g],
                        bias=bias_t[:, g * HT + h:g * HT + h + 1],
                        # h=0's i-gate is bf16 (unscaled); h>=1's is fp8
                        # with both operands pre-scaled by 16.
                        scale=(1.0 / (FP8_SCALE * FP8_SCALE))
                        if (g == 0 and h > 0) else 1.0,
                    )
                    return t

                # i, f, c~ first; the whole c_next/tanh chain runs while
                # the output gate's matmuls are still on the PE (gate-major
                # issue order puts o last).
                gi = _act_gate(0)
                gf = _act_gate(1)
                gc = _act_gate(2)

                t1 = epool.tile([128, 512], MM_DT, tag="t1", name=f"t1_{h}_{b2}")
                nc.vector.tensor_mul(t1[:], gi[:], gc[:])       # i * c~
                t2 = epool.tile([128, 512], MM_DT, tag="t2", name=f"t2_{h}_{b2}")
                nc.vector.tensor_mul(t2[:], gf[:], cp_tiles[h][:, cs])
                cn = epool.tile([128, 512], MM_DT, tag="cn", name=f"cn_{h}_{b2}")
                nc.vector.tensor_add(cn[:], t1[:], t2[:])
                nc.gpsimd.dma_start(c_nextT[hs, cs], cn[:])

                th = epool.tile([128, 512], MM_DT, tag="th", name=f"th_{h}_{b2}")
                nc.scalar.activation(th[:], cn[:], _TANH)

                go = _act_gate(3)
                hn = epool.tile([128, 512], MM_DT, tag="hn", name=f"hn_{h}_{b2}")
                nc.vector.tensor_mul(hn[:], go[:], th[:])
                nc.gpsimd.dma_start(h_nextT[hs, cs], hn[:])

            # h=0 rides the input-DMA ramp: every group needs all 8 k-tiles,
            # so widen to all 8 PSUM banks (4 gates x 2 batch halves) and
            # issue all four gates (i included, in bf16) k-major: the PE
            # consumes each k-tile right as it lands.
            psum0 = {b2: [_mk_psum(g, 0, b2) for g in range(NG)] for b2 in range(BT)}
            for k in range(KT):
                for g in range(NG):
                    for b2 in range(BT):
                        nc.tensor.matmul(
                            psum0[b2][g][:],
                            _lhsT(k, 0, g),
                            _rhs(k, b2),
                            start=(k == 0),
                            stop=(k == KT - 1),
                        )
            for b2 in range(BT):
                _elementwise(0, b2, psum0[b2])

            # h=1..2: per-(h,b2) 4-bank groups with b2 parity alternating
            # between the two bank sets, so each set's ACT drain overlaps
            # the other's matmuls. Both groups' fp8 i-gate batches run up
            # front (single fp8 mode-switch pair per h); the o gate stays
            # last so only ACT(o) + one mul trail the group's matmuls.
            for h in range(1, HT - 1):
                psum = {
                    b2: [_mk_psum(g, h, b2) for g in range(NG)] for b2 in range(BT)
                }
                for b2 in range(BT):
                    for kp in range(KP):
                        _mm_i(psum[b2][0][:], kp, h, b2)
                for b2 in range(BT):
                    for g in range(1, NG):
                        for k in range(KT):
                            nc.tensor.matmul(
                                psum[b2][g][:],
                                _lhsT(k, h, g),
                                _rhs(k, b2),
                                start=(k == 0),
                                stop=(k == KT - 1),
                            )
                    _elementwise(h, b2, psum[b2])

            # Last h-chunk: interleave f/c across the two batch halves so
            # the second group's sigmoid/tanh ACTs all fit before its
            # o-gate matmuls finish -- ACT is strict FIFO, and in group-
            # major order the last group's ~4.6us of ACT work would trail
            # its ~1.7us of o matmuls. With the interleave, only ACT(o) +
            # one DVE mul + the store follow the kernel's last matmul. All
            # of the last h's stores ride the sync ring so the exit barrier
            # isn't waiting on a late SWDGE receipt.
            h = HT - 1
            hs = slice(h * 128, (h + 1) * 128)
            psum = {
                b2: [_mk_psum(g, h, b2) for g in range(NG)] for b2 in range(BT)
            }
            for b2 in range(BT):
                for kp in range(KP):
                    _mm_i(psum[b2][0][:], kp, h, b2)
            for g in range(1, NG):
                for b2 in range(BT):
                    for k in range(KT):
                        nc.tensor.matmul(
                            psum[b2][g][:],
                            _lhsT(k, h, g),
                            _rhs(k, b2),
                            start=(k == 0),
                            stop=(k == KT - 1),
                        )

            def _act3(g, b2, lo, wd, tag):
                t = gpool.tile([128, wd], MM_DT, tag=tag, name=f"{tag}_{b2}_{lo}")
                nc.scalar.activation(
                    t[:], psum[b2][g][:, lo:lo + wd], _GATE_FN[g],
                    bias=bias_t[:, g * HT + h:g * HT + h + 1],
                    scale=(1.0 / (FP8_SCALE * FP8_SCALE)) if g == 0 else 1.0,
                )
                return t

            gi3 = [_act3(0, b2, 0, 512, "g0") for b2 in range(BT)]
            gf3 = [_act3(1, b2, 0, 512, "g1") for b2 in range(BT)]
            th3 = []
            for b2 in range(BT):
                cs = slice(b2 * 512, (b2 + 1) * 512)
                gc = _act3(2, b2, 0, 512, "g2")
                t1 = epool.tile([128, 512], MM_DT, tag="t1", name=f"t1_3_{b2}")
                nc.vector.tensor_mul(t1[:], gi3[b2][:], gc[:])
                t2 = epool.tile([128, 512], MM_DT, tag="t2", name=f"t2_3_{b2}")
                nc.vector.tensor_mul(t2[:], gf3[b2][:], cp_tiles[h][:, cs])
                cn = epool.tile([128, 512], MM_DT, tag="cn", name=f"cn_3_{b2}")
                nc.vector.tensor_add(cn[:], t1[:], t2[:])
                nc.sync.dma_start(c_nextT[hs, cs], cn[:])
                th = epool.tile([128, 512], MM_DT, tag="th", name=f"th_3_{b2}")
                nc.scalar.activation(th[:], cn[:], _TANH)
                th3.append(th)
            # b2=0's o path (its matmuls end one gate earlier)
            go = _act3(3, 0, 0, 512, "g3")
            hn = epool.tile([128, 512], MM_DT, tag="hn", name="hn_3_0")
            nc.vector.tensor_mul(hn[:], go[:], th3[0][:])
            nc.sync.dma_start(h_nextT[hs, 0:512], hn[:])
            # b2=1's o path, in two chunks so the first store issues early
            for c in range(2):
                lo = c * 256
                goc = _act3(3, 1, lo, 256, "g3")
                hnc = epool.tile([128, 256], MM_DT, tag="hn", name=f"hn_3_1_{c}")
                nc.vector.tensor_mul(hnc[:], goc[:], th3[1][:, lo:lo + 256])
                nc.sync.dma_start(h_nextT[hs, 512 + lo:512 + lo + 256], hnc[:])

    nc.compile()
    return nc


_NC_CACHE = None
_LAST_IN_MAPS = None


def kernel(x, h_prev, c_prev, W_i, b_i, W_f, b_f, W_c, b_c, W_o, b_o):
    global _NC_CACHE, _LAST_IN_MAPS
    if _NC_CACHE is None:
        _NC_CACHE = _build()
    nc = _NC_CACHE

    np_bf16 = mybir.dt.np(MM_DT)
    np_fp8 = mybir.dt.np(FP8)

    combT = np.concatenate([x, h_prev], axis=1).T          # (K, BATCH) f32
    comb8 = (combT * FP8_SCALE).astype(np_fp8)             # fp8, pre-scaled
    combT = combT.astype(np_bf16)

    wT4 = np.concatenate([W_i, W_f, W_c, W_o], axis=0).T   # (K, 4H): col g*512+h*128+c
    w0P = np.ascontiguousarray(
        wT4.reshape(K, NG, HT, 128)[:, :, 0, :].reshape(K, NG * 128)
    ).astype(np_bf16)
    wT3 = np.concatenate([W_f, W_c, W_o], axis=0).T        # (K, 3H)
    wHP = np.ascontiguousarray(
        wT3.reshape(KT, 128, 3, HT, 128).transpose(3, 1, 0, 2, 4)[1:]
        .reshape((HT - 1) * 128, KT * 3 * 128)
    ).astype(np_bf16)
    wiT = (W_i.T * FP8_SCALE).astype(np_fp8)               # (K, H) fp8
    wi8H = np.ascontiguousarray(
        wiT.reshape(KP, 2, 128, HT, 128).transpose(3, 2, 0, 1, 4)[1:]
        .reshape((HT - 1) * 128, KP * 256)
    )
    bias2d = np.ascontiguousarray(
        np.concatenate([b_i, b_f, b_c, b_o]).reshape(NG * HT, 128).T
    ).astype(np.float32)                                   # (128, 16)
    c_prevT = c_prev.T.astype(np_bf16)                     # (H, BATCH)

    in_maps = []
    for j in range(N_CORES):
        cols = slice(j * B, (j + 1) * B)
        xh8P = np.ascontiguousarray(
            comb8[:, cols].reshape(KP, 2, 128, B).transpose(0, 2, 1, 3)
            .reshape(KP * 128, 2 * B)
        )
        in_maps.append({
            "xhT": np.ascontiguousarray(combT[:, cols]),
            "xh8P": xh8P,
            "w0P": w0P,
            "wHP": wHP,
            "wi8H": wi8H,
            "bias2d": bias2d,
            "c_prevT": np.ascontiguousarray(c_prevT[:, cols]),
        })

    _LAST_IN_MAPS = in_maps
    try:
        res = run_bass_kernel_spmd(nc, in_maps, core_ids=list(range(N_CORES)))
    except Exception:
        # transient NRT_EXEC_UNIT_UNRECOVERABLE has been observed once on an
        # otherwise-correct NEFF; one retry is cheap insurance.
        res = run_bass_kernel_spmd(nc, in_maps, core_ids=list(range(N_CORES)))

    h_next = np.concatenate([r["h_nextT"].T for r in res.results], axis=0)
    c_next = np.concatenate([r["c_nextT"].T for r in res.results], axis=0)
    return (h_next.astype(np.float32), c_next.astype(np.float32))


# revision 24
# speedup vs baseline: 1.0226x; 1.0226x over previous
"""LSTM cell (batch 8192, input 512, hidden 512) on 8 Trainium2 NeuronCores.

Data-parallel over the batch dim: each core handles 1024 rows. Weights are
replicated. The host pre-transposes both matmul operands so the contraction
dim (fan_in = 1024) lands on SBUF partitions:

  gate.T[n, b] = sum_k W.T[k, n] * combined.T[k, b]     (matmul: lhsT.T @ rhs)

so the kernel computes everything in [hidden, batch] layout; gate biases
become per-partition vectors (free on the ACT activation op), and the host
transposes the outputs back after the gather.

The kernel is PE-bound, so precision is spent where the accuracy budget
allows:
 - f/c/o gate matmuls are bf16 (their fp8 error blows the 2e-2 gate: the
   f-gate error is amplified by c_prev, the c~-gate error passes straight
   through tanh into c_next).
 - The i-gate matmul for h>=1 runs fp8-e4m3 in DoubleRow mode (2 MACs per
   cell per cycle), halving its PE time. i only multiplies the bounded c~,
   so its fp8 error lands at ~1.7e-2 end to end (verified against the
   reference). Both operands are pre-scaled by 16 on the host to stay
   clear of the e4m3 subnormal region; the ACT sigmoid's free affine
   scale (1/256) undoes it. The fp8<->bf16 PE array mode switch costs a
   ~0.4us pipeline bubble, so all i-gate matmuls of one h-chunk are
   batched (one switch pair per h). The h=0 i-gate stays bf16: the h=0
   phase is DMA-bound and adding the fp8 operand stream to the ramp costs
   more than the fp8 matmul saves.
 - The elementwise tail, c_prev, and the outputs are bf16 (2x DVE rate,
   half the HBM traffic); the host widens outputs back to f32.

DMA scheduling is issuance-budgeted: a dma_start occupies its issuing
engine for ~0.65us, so the h=0 phase uses fine-grained per-k strips
(paced against the matmul ramp) while the h=1..3 weights arrive as one
contiguous block per h. Dummy matmuls on a memset tile pre-warm the PE's
HAM clock gate during the ramp. Outputs ride the gpsimd SWDGE ring except
the final h-chunk's, which use sync so the exit barrier isn't waiting on
a late SWDGE receipt.
"""

import numpy as np

import concourse.bacc as bacc
import concourse.bass as bass
import concourse.mybir as mybir
from concourse import tile
from concourse.bass_utils import run_bass_kernel_spmd

N_CORES = 8
BATCH = 8192
B = BATCH // N_CORES  # 1024 batch rows per core
K = 1024              # fan_in = input_dim + hidden_dim
H = 512               # hidden dim
NG = 4                # gates: i, f, c, o
KT = K // 128         # 8 contraction tiles
KP = KT // 2          # 4 fp8 DoubleRow k-pair tiles
HT = H // 128         # 4 hidden chunks per gate
BT = B // 512         # 2 batch halves (PSUM free-dim limit is 512 f32)

MM_DT = mybir.dt.bfloat16
FP8 = mybir.dt.float8e4
F32 = mybir.dt.float32
FP8_SCALE = 16.0      # host-side pre-scale on both i-gate operands

_SIG = mybir.ActivationFunctionType.Sigmoid
_TANH = mybir.ActivationFunctionType.Tanh
_DR = mybir.MatmulPerfMode.DoubleRow
# gate order: i, f, c, o
_GATE_FN = [_SIG, _SIG, _TANH, _SIG]


def _build():
    nc = bacc.Bacc(
        "TRN2",
        target_bir_lowering=False,
        debug=False,
        num_devices=N_CORES,
    )

    xhT = nc.dram_tensor("xhT", [K, B], MM_DT, kind="ExternalInput")
    # fp8 activations, k-pair-interleaved: row kp*128+p, col s*B+b holds
    # 16*combined.T[(2kp+s)*128+p, b] so each pair tile is contiguous.
    xh8P = nc.dram_tensor("xh8P", [KP * 128, 2 * B], FP8, kind="ExternalInput")
    # h=0 weights, all four gates, one contiguous 128KB strip per k:
    # row k*128+p, col g*128+c.
    w0P = nc.dram_tensor("w0P", [KT * 128, NG * 128], MM_DT, kind="ExternalInput")
    # h>=1 f/c/o weights, one contiguous 768KB block per h:
    # row (h-1)*128+p, col k*384 + (g-1)*128 + c.
    wHP = nc.dram_tensor(
        "wHP", [(HT - 1) * 128, KT * 3 * 128], MM_DT, kind="ExternalInput",
    )
    # h>=1 fp8 i-gate weights, one contiguous 128KB block per h:
    # row (h-1)*128+p, col kp*256 + s*128 + c.
    wi8H = nc.dram_tensor(
        "wi8H", [(HT - 1) * 128, KP * 256], FP8, kind="ExternalInput",
    )
    bias2d = nc.dram_tensor("bias2d", [128, NG * HT], F32, kind="ExternalInput")
    c_prevT = nc.dram_tensor("c_prevT", [H, B], MM_DT, kind="ExternalInput")
    h_nextT = nc.dram_tensor("h_nextT", [H, B], MM_DT, kind="ExternalOutput")
    c_nextT = nc.dram_tensor("c_nextT", [H, B], MM_DT, kind="ExternalOutput")

    with tile.TileContext(nc) as tc:
        with (
            tc.tile_pool(name="wts", bufs=1) as wpool,
            tc.tile_pool(name="acts", bufs=1) as apool,
            tc.tile_pool(name="cprev", bufs=1) as cpool,
            tc.tile_pool(name="gates", bufs=3) as gpool,
            tc.tile_pool(name="ew", bufs=3) as epool,
            tc.tile_pool(name="psum", bufs=1, space="PSUM") as pspool,
        ):
            xh_tiles = [None] * KT          # [k] -> [128, B] bf16 (k>=1)
            xh0_half = [None, None]         # k=0 b2 halves [128, 512]
            x8_tiles = [None] * KP          # [kp] -> [128, 2, B] fp8
            w0_tiles = [None] * KT          # [k] -> [128, 512] bf16 (h=0)
            wH_tiles = [None] * HT          # [h] -> [128, KT*384] bf16 (h>=1)
            wi_tiles = [None] * HT          # [h] -> [128, KP*2, 128] fp8
            cp_tiles = [None] * HT

            def _load_w0(k, eng=None):
                wt = wpool.tile([128, NG * 128], MM_DT, tag=f"w0_{k}", name=f"w0_{k}")
                (eng or nc.sync).dma_start(wt[:], w0P[k * 128:(k + 1) * 128, :])
                w0_tiles[k] = wt

            def _load_wH(h):
                wt = wpool.tile([128, KT * 384], MM_DT, tag=f"wH{h}", name=f"wH{h}")
                nc.sync.dma_start(wt[:], wHP[(h - 1) * 128:h * 128, :])
                wH_tiles[h] = wt

            def _load_wi(h):
                wt = wpool.tile(
                    [128, KP * 2, 128], FP8, tag=f"wi{h}", name=f"wi{h}",
                )
                nc.sync.dma_start(wt[:], wi8H[(h - 1) * 128:h * 128, :])
                wi_tiles[h] = wt

            def _load_x8(kp):
                xt = apool.tile([128, 2, B], FP8, tag=f"x8_{kp}", name=f"x8_{kp}")
                nc.scalar.dma_start(xt[:], xh8P[kp * 128:(kp + 1) * 128, :])
                x8_tiles[kp] = xt

            def _load_cp(h):
                ct = cpool.tile([128, B], MM_DT, tag=f"cp{h}", name=f"cp{h}")
                nc.sync.dma_start(ct[:], c_prevT[h * 128:(h + 1) * 128, :])
                cp_tiles[h] = ct

            # ---- input DMA schedule ------------------------------------
            # scalar ring: k=0/1 h=0 weight strips (so the first matmul's
            #   two dependencies land in parallel), then the fp8 activation
            #   pair tiles (first used at h=1).
            # sync ring: h=0 k-stream (weights + activations, k-major,
            #   paced against the matmul ramp), with the h=1..3 per-h
            #   weight blocks and c_prev tiles threaded through at the
            #   points where issuance slack exists.
            _load_w0(0, eng=nc.scalar)
            _load_w0(1, eng=nc.scalar)
            for kp in range(KP):
                _load_x8(kp)

            for b2 in range(BT):
                xt = apool.tile(
                    [128, 512], MM_DT, tag=f"xh0_{b2}", name=f"xh0_{b2}",
                )
                nc.sync.dma_start(xt[:], xhT[0:128, b2 * 512:(b2 + 1) * 512])
                xh0_half[b2] = xt
            bias_t = wpool.tile([128, NG * HT], F32, tag="bias", name="bias_t")
            nc.sync.dma_start(bias_t[:], bias2d[:])
            for k in range(1, KT):
                xt = apool.tile([128, B], MM_DT, tag=f"xh{k}", name=f"xh{k}")
                nc.sync.dma_start(xt[:], xhT[k * 128:(k + 1) * 128, :])
                xh_tiles[k] = xt
                if k >= 2:
                    _load_w0(k)
            # The ring is FIFO: the big h>=1 blocks go strictly AFTER the
            # h=0 k-stream so they can't delay it, and in consumption
            # order. They all land with multi-us margin.
            _load_cp(0)
            _load_wi(1)
            _load_wH(1)
            _load_cp(1)
            _load_wi(2)
            _load_wH(2)
            _load_cp(2)
            _load_wi(3)
            _load_wH(3)
            _load_cp(3)

            # PE pre-warm: the HAM clock gate holds the PE at 1.2 GHz until
            # it has seen ~3.4us of sustained matmul activity. Dummy matmuls
            # on a memset tile during the input-DMA ramp start that clock
            # early, so the real matmul stream runs warm (2.4 GHz) almost
            # from its first instruction.
            warm_t = apool.tile([128, 512], MM_DT, tag="warm", name="warm_t")
            nc.vector.memset(warm_t[:], 0)
            warm_ps = pspool.tile([128, 512], F32, tag="ps0_0", name="warm_ps")
            for _ in range(5):
                nc.tensor.matmul(
                    warm_ps[:], warm_t[:, 0:128], warm_t[:],
                    start=True, stop=True,
                )

            def _rhs(k, b2):
                if k == 0:
                    return xh0_half[b2][:]
                return xh_tiles[k][:, b2 * 512:(b2 + 1) * 512]

            def _rhs8(kp, b2):
                return x8_tiles[kp][:, :, b2 * 512:(b2 + 1) * 512]

            def _lhsT(k, h, g):
                if h == 0:
                    return w0_tiles[k][:, g * 128:(g + 1) * 128]
                # bf16 gates only for h>=1: g in {1, 2, 3}
                off = k * 384 + (g - 1) * 128
                return wH_tiles[h][:, off:off + 128]

            def _mm_i(psum_i, kp, h, b2):
                nc.tensor.matmul(
                    psum_i,
                    wi_tiles[h][:, 2 * kp:2 * kp + 2, :],
                    _rhs8(kp, b2),
                    start=(kp == 0),
                    stop=(kp == KP - 1),
                    perf_mode=_DR,
                )

            def _mk_psum(g, h, b2):
                return pspool.tile(
                    [128, 512], F32,
                    tag=f"ps{g}_{b2 % 2}", name=f"ps{g}_{h}_{b2}",
                )

            def _elementwise(h, b2, psum):
                """Activations + LSTM cell tail for one (h, b2) group."""
                hs = slice(h * 128, (h + 1) * 128)
                cs = slice(b2 * 512, (b2 + 1) * 512)

                def _act_gate(g):
                    t = gpool.tile(
                        [128, 512], MM_DT, tag=f"g{g}", name=f"g{g}_{h}_{b2}",
                    )
                    nc.scalar.activation(
                        t[:], psum[g][:], _GATE_FN[g],
                        bias=bias_t[:, g * HT + h:g * HT + h + 1],
                        # h=0's i-gate is bf16 (unscaled); h>=1's is fp8
                        # with both operands pre-scaled by 16.
                        scale=(1.0 / (FP8_SCALE * FP8_SCALE))
                        if (g == 0 and h > 0) else 1.0,
                    )
                    return t

                # i, f, c~ first; the whole c_next/tanh chain runs while
                # the output gate's matmuls are still on the PE (gate-major
                # issue order puts o last).
                gi = _act_gate(0)
                gf = _act_gate(1)
                gc = _act_gate(2)

                t1 = epool.tile([128, 512], MM_DT, tag="t1", name=f"t1_{h}_{b2}")
                nc.vector.tensor_mul(t1[:], gi[:], gc[:])       # i * c~
                t2 = epool.tile([128, 512], MM_DT, tag="t2", name=f"t2_{h}_{b2}")
                nc.vector.tensor_mul(t2[:], gf[:], cp_tiles[h][:, cs])
                cn = epool.tile([128, 512], MM_DT, tag="cn", name=f"cn_{h}_{b2}")
                nc.vector.tensor_add(cn[:], t1[:], t2[:])
                nc.gpsimd.dma_start(c_nextT[hs, cs], cn[:])

                th = epool.tile([128, 512], MM_DT, tag="th", name=f"th_{h}_{b2}")
                nc.scalar.activation(th[:], cn[:], _TANH)

                go = _act_gate(3)
                hn = epool.tile([128, 512], MM_DT, tag="hn", name=f"hn_{h}_{b2}")
                nc.vector.tensor_mul(hn[:], go[:], th[:])
                nc.gpsimd.dma_start(h_nextT[hs, cs], hn[:])

            # h=0 rides the input-DMA ramp: every group needs all 8 k-tiles,
            # so widen to all 8 PSUM banks (4 gates x 2 batch halves) and
            # issue all four gates (i included, in bf16) k-major: the PE
            # consumes each k-tile right as it lands.
            psum0 = {b2: [_mk_psum(g, 0, b2) for g in range(NG)] for b2 in range(BT)}
            for k in range(KT):
                for g in range(NG):
                    for b2 in range(BT):
                        nc.tensor.matmul(
                            psum0[b2][g][:],
                            _lhsT(k, 0, g),
                            _rhs(k, b2),
                            start=(k == 0),
                            stop=(k == KT - 1),
                        )
            for b2 in range(BT):
                _elementwise(0, b2, psum0[b2])

            # h=1..2: per-(h,b2) 4-bank groups with b2 parity alternating
            # between the two bank sets, so each set's ACT drain overlaps
            # the other's matmuls. Both groups' fp8 i-gate batches run up
            # front (single fp8 mode-switch pair per h); the o gate stays
            # last so only ACT(o) + one mul trail the group's matmuls.
            for h in range(1, HT - 1):
                psum = {
                    b2: [_mk_psum(g, h, b2) for g in range(NG)] for b2 in range(BT)
                }
                for b2 in range(BT):
                    for kp in range(KP):
                        _mm_i(psum[b2][0][:], kp, h, b2)
                for b2 in range(BT):
                    for g in range(1, NG):
                        for k in range(KT):
                            nc.tensor.matmul(
                                psum[b2][g][:],
                                _lhsT(k, h, g),
                                _rhs(k, b2),
                                start=(k == 0),
                                stop=(k == KT - 1),
                            )
                    _elementwise(h, b2, psum[b2])

            # Last h-chunk: interleave f/c across the two batch halves so
            # the second group's sigmoid/tanh ACTs all fit before its
            # o-gate matmuls finish -- ACT is strict FIFO, and in group-
            # major order the last group's ~4.6us of ACT work would trail
            # its ~1.7us of o matmuls. With the interleave, only ACT(o) +
            # one DVE mul + the store follow the kernel's last matmul. All
            # of the last h's stores ride the sync ring so the exit barrier
            # isn't waiting on a late SWDGE receipt.
            h = HT - 1
            hs = slice(h * 128, (h + 1) * 128)
            psum = {
                b2: [_mk_psum(g, h, b2) for g in range(NG)] for b2 in range(BT)
            }
            for b2 in range(BT):
                for kp in range(KP):
                    _mm_i(psum[b2][0][:], kp, h, b2)
            for g in range(1, NG):
                for b2 in range(BT):
                    for k in range(KT):
                        nc.tensor.matmul(
                            psum[b2][g][:],
                            _lhsT(k, h, g),
                            _rhs(k, b2),
                            start=(k == 0),
                            stop=(k == KT - 1),
                        )

            def _act3(g, b2, lo, wd, tag):
                t = gpool.tile([128, wd], MM_DT, tag=tag, name=f"{tag}_{b2}_{lo}")
                nc.scalar.activation(
                    t[:], psum[b2][g][:, lo:lo + wd], _GATE_FN[g],
                    bias=bias_t[:, g * HT + h:g * HT + h + 1],
                    scale=(1.0 / (FP8_SCALE * FP8_SCALE)) if g == 0 else 1.0,
                )
                return t

            gi3 = [_act3(0, b2, 0, 512, "g0") for b2 in range(BT)]
            gf3 = [_act3(1, b2, 0, 512, "g1") for b2 in range(BT)]
            th3 = []
            for b2 in range(BT):
                cs = slice(b2 * 512, (b2 + 1) * 512)
                gc = _act3(2, b2, 0, 512, "g2")
                t1 = epool.tile([128, 512], MM_DT, tag="t1", name=f"t1_3_{b2}")
                nc.vector.tensor_mul(t1[:], gi3[b2][:], gc[:])
                t2 = epool.tile([128, 512], MM_DT, tag="t2", name=f"t2_3_{b2}")
                nc.vector.tensor_mul(t2[:], gf3[b2][:], cp_tiles[h][:, cs])
                cn = epool.tile([128, 512], MM_DT, tag="cn", name=f"cn_3_{b2}")
                nc.vector.tensor_add(cn[:], t1[:], t2[:])
                nc.sync.dma_start(c_nextT[hs, cs], cn[:])
                th = epool.tile([128, 512], MM_DT, tag="th", name=f"th_3_{b2}")
                nc.scalar.activation(th[:], cn[:], _TANH)
                th3.append(th)
            # b2=0's o path (its matmuls end one gate earlier)
            go = _act3(3, 0, 0, 512, "g3")
            hn = epool.tile([128, 512], MM_DT, tag="hn", name="hn_3_0")
            nc.vector.tensor_mul(hn[:], go[:], th3[0][:])
            nc.sync.dma_start(h_nextT[hs, 0:512], hn[:])
            # b2=1's o path, in two chunks so the first store issues early
            for c in range(2):
                lo = c * 256
                goc = _act3(3, 1, lo, 256, "g3")
                hnc = epool.tile([128, 256], MM_DT, tag="hn", name=f"hn_3_1_{c}")
                nc.vector.tensor_mul(hnc[:], goc[:], th3[1][:, lo:lo + 256])
                nc.sync.dma_start(h_nextT[hs, 512 + lo:512 + lo + 256], hnc[:])

    nc.compile()
    return nc


_NC_CACHE = None
_LAST_IN_MAPS = None


def kernel(x, h_prev, c_prev, W_i, b_i, W_f, b_f, W_c, b_c, W_o, b_o):
    global _NC_CACHE, _LAST_IN_MAPS
    if _NC_CACHE is None:
        _NC_CACHE = _build()
    nc = _NC_CACHE

    np_bf16 = mybir.dt.np(MM_DT)
    np_fp8 = mybir.dt.np(FP8)

    combT = np.concatenate([x, h_prev], axis=1).T          # (K, BATCH) f32
    comb8 = (combT * FP8_SCALE).astype(np_fp8)             # fp8, pre-scaled
    combT = combT.astype(np_bf16)

    wT4 = np.concatenate([W_i, W_f, W_c, W_o], axis=0).T   # (K, 4H): col g*512+h*128+c
    w0P = np.ascontiguousarray(
        wT4.reshape(K, NG, HT, 128)[:, :, 0, :].reshape(K, NG * 128)
    ).astype(np_bf16)
    wT3 = np.concatenate([W_f, W_c, W_o], axis=0).T        # (K, 3H)
    wHP = np.ascontiguousarray(
        wT3.reshape(KT, 128, 3, HT, 128).transpose(3, 1, 0, 2, 4)[1:]
        .reshape((HT - 1) * 128, KT * 3 * 128)
    ).astype(np_bf16)
    wiT = (W_i.T * FP8_SCALE).astype(np_fp8)               # (K, H) fp8
    wi8H = np.ascontiguousarray(
        wiT.reshape(KP, 2, 128, HT, 128).transpose(3, 2, 0, 1, 4)[1:]
        .reshape((HT - 1) * 128, KP * 256)
    )
    bias2d = np.ascontiguousarray(
        np.concatenate([b_i, b_f, b_c, b_o]).reshape(NG * HT, 128).T
    ).astype(np.float32)                                   # (128, 16)
    c_prevT = c_prev.T.astype(np_bf16)                     # (H, BATCH)

    in_maps = []
    for j in range(N_CORES):
        cols = slice(j * B, (j + 1) * B)
        xh8P = np.ascontiguousarray(
            comb8[:, cols].reshape(KP, 2, 128, B).transpose(0, 2, 1, 3)
            .reshape(KP * 128, 2 * B)
        )
        in_maps.append({
            "xhT": np.ascontiguousarray(combT[:, cols]),
            "xh8P": xh8P,
            "w0P": w0P,
            "wHP": wHP,
            "wi8H": wi8H,
            "bias2d": bias2d,
            "c_prevT": np.ascontiguousarray(c_prevT[:, cols]),
        })

    _LAST_IN_MAPS = in_maps
    try:
        res = run_bass_kernel_spmd(nc, in_maps, core_ids=list(range(N_CORES)))
    except Exception:
        # transient NRT_EXEC_UNIT_UNRECOVERABLE has been observed once on an
        # otherwise-correct NEFF; one retry is cheap insurance.
        res = run_bass_kernel_spmd(nc, in_maps, core_ids=list(range(N_CORES)))

    h_next = np.concatenate([r["h_nextT"].T for r in res.results], axis=0)
    c_next = np.concatenate([r["c_nextT"].T for r in res.results], axis=0)
    return (h_next.astype(np.float32), c_next.astype(np.float32))
